# revision 28
# baseline (speedup 1.0000x reference)
"""Trainium2 Bass kernel: 6-layer dense transformer (B=2, N=2048, E=768, H=12,
FF=3072, ALiBi, causal), full inputs -> full output, distributed over 8 cores.

Sharding: balanced sequence-parallel. Core i owns 512 tokens: batch0 chunk i
(tokens 256i..256i+256) and batch1 chunk 7-i. Per layer, each core computes
local QKV, AllGathers K (fp32r, position-augmented) and V (bf16) across the 8
cores, runs flash-style attention on its 512 queries, then output projection
and the FFN locally. Activations live transposed ([channel, token]) in SBUF;
LayerNorm reductions run as ones-matmuls on the PE.

ALiBi + softmax max-subtraction are folded into the attention score matmul via
two augmented contraction rows carrying exact fp32 positions:
  S = (q*s + bq*s) . (k + bk) + slope*(kpos - qpos)
The per-query -slope*qpos term cancels in softmax but keeps every exp argument
<= O(1) so no row-max pass is needed. exp(S) is taken directly out of PSUM.

SPMD uniformity: every core runs 7 "off-diagonal" attention slots (key blocks
from other cores, no causal masking needed) + 2 local diagonal blocks (static
causal affine_select). Which (rank, half) a slot reads is per-core DATA: the
readback DMAs use register-based dynamic offsets, and two "kill rows" in the
augmented matmul zero out the scores of the inactive query half.
"""

import math
import os

import numpy as np
import ml_dtypes

DEPTH, EMB, HEADS = 6, 768, 12
B, N = 2, 2048
H, DH = 12, 64
FF = 3072
LN_EPS = 1e-6
N_CORES = 8
LOC = 512           # local tokens per core (256 per batch)
HALF = 256
SCALE = DH ** -0.5
KILL = -60000.0

_DEPTH = int(os.environ.get("BASS_DEPTH", DEPTH))

bf16 = ml_dtypes.bfloat16


def _slopes(n):
    def p2(n):
        start = 2 ** (-(2 ** (-(math.log2(n) - 3))))
        return [start * start ** i for i in range(n)]
    if math.log2(n).is_integer():
        return p2(n)
    c = 2 ** math.floor(math.log2(n))
    return p2(c) + _slopes(2 * c)[0::2][: n - c]


SLOPES = np.asarray(_slopes(H), np.float64)

# ---------------------------------------------------------------------------
# device program
# ---------------------------------------------------------------------------

_CACHED = {}


def _build():
    if "nc" in _CACHED:
        return _CACHED["nc"]
    import concourse.bacc as bacc
    import concourse.bass as bass
    import concourse.mybir as mybir
    import concourse.tile as tile
    from concourse.ordered_set import OrderedSet

    AF = mybir.ActivationFunctionType
    ALU = mybir.AluOpType
    F32 = mybir.dt.float32
    F32R = mybir.dt.float32r
    BF16 = mybir.dt.bfloat16
    I32 = mybir.dt.int32
    POOL = mybir.EngineType.Pool

    nc = bacc.Bacc("TRN2", num_devices=N_CORES)

    # ---- I/O ----
    xT = nc.dram_tensor("xT", [128, 6, LOC], F32, kind="ExternalInput")
    wqk = nc.dram_tensor("wqk", [_DEPTH, EMB, 2 * EMB], BF16, kind="ExternalInput")
    wv = nc.dram_tensor("wv", [_DEPTH, EMB, EMB], BF16, kind="ExternalInput")
    wo = nc.dram_tensor("wo", [_DEPTH, EMB, EMB], BF16, kind="ExternalInput")
    w1 = nc.dram_tensor("w1", [_DEPTH, EMB, FF], BF16, kind="ExternalInput")
    w2 = nc.dram_tensor("w2", [_DEPTH, FF, EMB], BF16, kind="ExternalInput")
    bq_s = nc.dram_tensor("bq_s", [128, _DEPTH, 6], F32, kind="ExternalInput")
    bk_c = nc.dram_tensor("bk_c", [128, _DEPTH, 6], F32, kind="ExternalInput")
    bo_c = nc.dram_tensor("bo_c", [128, _DEPTH, 6], F32, kind="ExternalInput")
    bv_h = nc.dram_tensor("bv_h", [64, _DEPTH, H], F32, kind="ExternalInput")
    ln1s_c = nc.dram_tensor("ln1s_c", [128, _DEPTH, 6], F32, kind="ExternalInput")
    ln1b_c = nc.dram_tensor("ln1b_c", [128, _DEPTH, 6], F32, kind="ExternalInput")
    ln2s_c = nc.dram_tensor("ln2s_c", [128, _DEPTH, 6], F32, kind="ExternalInput")
    ln2b_c = nc.dram_tensor("ln2b_c", [128, _DEPTH, 6], F32, kind="ExternalInput")
    lnf_c = nc.dram_tensor("lnf_c", [128, 2, 6], F32, kind="ExternalInput")
    qaug = nc.dram_tensor("qaug", [H, 2, LOC], F32R, kind="ExternalInput")
    qkill = nc.dram_tensor("qkill", [2, LOC], F32R, kind="ExternalInput")
    kaug = nc.dram_tensor("kaug", [2, LOC], F32R, kind="ExternalInput")
    flagrows = nc.dram_tensor("flagrows", [7, 2, 1024], F32R, kind="ExternalInput")
    slotreg = nc.dram_tensor("slotreg", [1, 28], I32, kind="ExternalInput")
    ones_col = nc.dram_tensor("ones_col", [128, 1], F32R, kind="ExternalInput")
    ones_row = nc.dram_tensor("ones_row", [1, 256], F32R, kind="ExternalInput")
    out_d = nc.dram_tensor("out", [128, 6, LOC], F32, kind="ExternalOutput")
    DBG = bool(int(os.environ.get("BASS_DEBUG", "0")))
    PHASES = os.environ.get("BASS_PHASES", "all")
    if DBG:
        dbg_yT = nc.dram_tensor("dbg_yT", [128, 6, LOC], BF16, kind="ExternalOutput")
        dbg_Ka = nc.dram_tensor("dbg_Ka", [66, 2 * H * HALF], F32R, kind="ExternalOutput")
        dbg_qa = nc.dram_tensor("dbg_qa", [68, 2 * LOC], F32R, kind="ExternalOutput")
        dbg_oT = nc.dram_tensor("dbg_oT", [128, 6, LOC], BF16, kind="ExternalOutput")
        dbg_hT = nc.dram_tensor("dbg_hT", [128, 6, LOC], F32, kind="ExternalOutput")
        dbg_ex = nc.dram_tensor("dbg_ex", [128, 2 * 1024], BF16, kind="ExternalOutput")
        dbg_yT0 = nc.dram_tensor(
            "dbg_yT0", [128, 6, LOC],
            F32 if os.environ.get("BASS_YT_F32") == "1" else BF16,
            kind="ExternalOutput")
        dbg_kr = nc.dram_tensor("dbg_kr", [68, 1024], F32R, kind="ExternalOutput")

    from contextlib import ExitStack

    with tile.TileContext(nc) as tc:
        with ExitStack() as ctx:
            ctx.enter_context(nc.allow_low_precision(
                reason="fp32r rounding of stats/normalizers is intentional"))
            pp = ctx.enter_context(tc.tile_pool(name="persist", bufs=1))
            wqp = ctx.enter_context(tc.tile_pool(name="wq", bufs=4))
            wvp = ctx.enter_context(tc.tile_pool(name="wvp", bufs=7))
            wop = ctx.enter_context(tc.tile_pool(name="wop", bufs=4))
            w1p = ctx.enter_context(tc.tile_pool(name="w1p", bufs=4))
            w2p = ctx.enter_context(tc.tile_pool(name="w2p", bufs=4))
            kresp = ctx.enter_context(tc.tile_pool(name="kres", bufs=2))
            vresp = ctx.enter_context(tc.tile_pool(name="vres", bufs=2))
            sqp = ctx.enter_context(tc.tile_pool(name="sqp", bufs=3))
            tmpp = ctx.enter_context(tc.tile_pool(name="tmpp", bufs=3))
            expp = ctx.enter_context(tc.tile_pool(name="expp", bufs=2))
            gp = ctx.enter_context(tc.tile_pool(name="gp", bufs=3))
            stg = ctx.enter_context(tc.tile_pool(name="stg", bufs=2))
            smal = ctx.enter_context(tc.tile_pool(name="smal", bufs=4))
            ps = ctx.enter_context(tc.tile_pool(name="ps", bufs=4, space="PSUM"))
            dram = ctx.enter_context(tc.tile_pool(name="dram", bufs=2, space="DRAM"))
            # ---- persistent tiles ----
            hT = pp.tile([128, 6, LOC], F32R, name="hT")
            yT = pp.tile([128, 6, LOC],
                         F32 if os.environ.get("BASS_YT_F32") == "1" else BF16,
                         name="yT")
            oT = pp.tile([128, 6, LOC], BF16, name="oT")
            Ka = pp.tile([66, 2, H, HALF], F32R, name="Ka")
            Va = pp.tile([128, 2, H, 2, 65], BF16, name="Va")
            qa = [pp.tile([68, LOC], F32R, name=f"qa{h}") for h in range(H)]
            ones128 = pp.tile([128, 1], F32R, name="ones128")
            onesrow = pp.tile([1, 256], F32R, name="onesrow")
            bqs_sb = pp.tile([128, _DEPTH, 6], F32, name="bqs_sb")
            bkc_sb = pp.tile([128, _DEPTH, 6], F32, name="bkc_sb")
            boc_sb = pp.tile([128, _DEPTH, 6], F32, name="boc_sb")
            bvh_sb = pp.tile([64, _DEPTH, H], F32, name="bvh_sb")
            l1s_sb = pp.tile([128, _DEPTH, 6], F32, name="l1s_sb")
            l1b_sb = pp.tile([128, _DEPTH, 6], F32, name="l1b_sb")
            l2s_sb = pp.tile([128, _DEPTH, 6], F32, name="l2s_sb")
            l2b_sb = pp.tile([128, _DEPTH, 6], F32, name="l2b_sb")
            lnf_sb = pp.tile([128, 2, 6], F32, name="lnf_sb")

            dma = nc.sync.dma_start
            act = nc.scalar.activation
            vec = nc.vector

            # ---- init: constants, tables, registers ----
            for c in range(6):
                xt = tmpp.tile([128, LOC], F32, name="lntmp", tag="lntmp")
                dma(out=xt[:], in_=xT[:, c, :])
                vec.tensor_copy(out=hT[:, c, :], in_=xt[:])
            for t, src in [
                (bqs_sb, bq_s), (bkc_sb, bk_c), (boc_sb, bo_c), (bvh_sb, bv_h),
                (l1s_sb, ln1s_c), (l1b_sb, ln1b_c), (l2s_sb, ln2s_c),
                (l2b_sb, ln2b_c), (lnf_sb, lnf_c),
            ]:
                dma(out=t[:], in_=src[:, :, :])
            dma(out=ones128[:], in_=ones_col[:, :])
            dma(out=onesrow[:], in_=ones_row[:, :])
            if os.environ.get("BASS_NO_VAMEMSET", "0") != "1":
                vec.memset(Va[:, :, :, :, 64:65], 1.0)
            if os.environ.get("BASS_NO_AUG") != "1":
                # aug rows: q side (slope, -slope*qpos) rows 64:66, kill 66:68
                for h in range(H):
                    dma(out=qa[h][64:66, :], in_=qaug[h, :, :])
                    dma(out=qa[h][66:68, :], in_=qkill[:, :])
                # k side: (kpos, ones) rows 64:66, replicated per head
                for h in range(H):
                    dma(out=Ka[64:66, :, h, :],
                        in_=kaug[:, :].rearrange("r (f t) -> r f t", f=2))
            # slot registers (gpsimd/Pool only)
            sreg_sb = pp.tile([1, 28], I32, name="sreg_sb")
            dma(out=sreg_sb[:], in_=slotreg[:, :])
            rvs = []
            NREG = 0 if os.environ.get("BASS_NO_REGS") == "1" else 28
            for k in range(NREG):
                reg = nc.alloc_registers(engines=(POOL,))
                nc.gpsimd.reg_load(reg[POOL], sreg_sb[0:1, k:k + 1])
                rvs.append(nc.snap(reg, engines=OrderedSet([POOL]), donate=True))
            if NREG == 0:
                rvs = [0] * 28
            rv_krow = rvs[0:7]
            rv_kcol = rvs[7:14]
            rv_vrow = rvs[14:21]
            rv_vcol = rvs[21:28]

            def ln_into(dst, s_col, b_col, out_dtype_bf16=True):
                """LayerNorm of hT -> dst (per-channel scale/bias cols)."""
                st = ps.tile([128, 1024], F32, name="ln_stats", tag="psA")
                for c in range(6):
                    sq = sqp.tile([128, LOC], F32R, name="sq")
                    vec.tensor_tensor(out=sq[:], in0=hT[:, c, :], in1=hT[:, c, :],
                                      op=ALU.mult)
                    nc.tensor.matmul(st[0:1, 0:512], ones128[:], hT[:, c, :],
                                     start=(c == 0), stop=(c == 5))
                    nc.tensor.matmul(st[0:1, 512:1024], ones128[:], sq[:],
                                     start=(c == 0), stop=(c == 5))
                m_sb = smal.tile([1, LOC], F32R, name="m_sb", tag="smal")
                vec.tensor_scalar(out=m_sb[:], in0=st[0:1, 0:512],
                                  scalar1=1.0 / EMB, scalar2=None, op0=ALU.mult)
                msq = smal.tile([1, LOC], F32, name="msq", tag="smal")
                vec.tensor_tensor(out=msq[:], in0=m_sb[:], in1=m_sb[:], op=ALU.mult)
                var = smal.tile([1, LOC], F32, name="var", tag="smal")
                vec.tensor_scalar(out=var[:], in0=st[0:1, 512:1024],
                                  scalar1=1.0 / EMB, scalar2=LN_EPS,
                                  op0=ALU.mult, op1=ALU.add)
                vec.tensor_tensor(out=var[:], in0=var[:], in1=msq[:], op=ALU.subtract)
                lnv = smal.tile([1, LOC], F32, name="lnv", tag="smal")
                act(lnv[:], var[:], AF.Ln, bias=0.0, scale=1.0)
                rstd = smal.tile([1, LOC], F32R, name="rstd", tag="smal")
                act(rstd[:], lnv[:], AF.Exp, bias=0.0, scale=-0.5)
                mrstd = smal.tile([1, LOC], F32R, name="mrstd", tag="smal")
                vec.tensor_tensor(out=mrstd[:], in0=m_sb[:], in1=rstd[:], op=ALU.mult)
                bc = ps.tile([128, 1024], F32, name="ln_bc", tag="psA")
                nc.tensor.matmul(bc[:, 0:512], onesrow[0:1, 0:128], rstd[:], start=True,
                                 stop=True)
                nc.tensor.matmul(bc[:, 512:1024], onesrow[0:1, 128:256], mrstd[:], start=True,
                                 stop=True)
                for c in range(6):
                    tmp = tmpp.tile([128, LOC], F32, name="lntmp", tag="lntmp")
                    vec.tensor_tensor(out=tmp[:], in0=hT[:, c, :], in1=bc[:, 0:512],
                                      op=ALU.mult)
                    vec.tensor_tensor(out=tmp[:], in0=tmp[:], in1=bc[:, 512:1024],
                                      op=ALU.add)
                    if os.environ.get("BASS_LN_FLOATSB") == "1":
                        act(dst[:, c, :], tmp[:], AF.Identity,
                            bias=0.0, scale=1.0)
                    else:
                        act(dst[:, c, :], tmp[:], AF.Identity,
                            bias=b_col(c), scale=s_col(c))

            kv_k_out_prev = None
            for l in range(_DEPTH):
                # ===== LN1 =====
                ln_into(yT, lambda c: l1s_sb[:, l, c:c + 1],
                        lambda c: l1b_sb[:, l, c:c + 1])

                if DBG and l == 0:
                    if os.environ.get("BASS_DUMP_GP") == "1":
                        nc.gpsimd.dma_start(out=dbg_yT0[:, :, :], in_=yT[:])
                    else:
                        dma(out=dbg_yT0[:, :, :], in_=yT[:])
                    for c in range(6):
                        yc = tmpp.tile([128, LOC], F32, name="lntmp", tag="lntmp")
                        vec.tensor_copy(out=yc[:], in_=yT[:, c, :])
                        dma(out=dbg_hT[:, c, :], in_=yc[:])
                if PHASES == "ln":
                    continue
                # ===== QKV: K part (heads -> Ka), kick AG-K =====
                def load_wq(m):
                    wt = wqp.tile([128, 6, 128], BF16, name="wqk_t", tag="wqk")
                    nc.gpsimd.dma_start(
                        out=wt[:], in_=wqk[l, :, 128 * m:128 * (m + 1)]
                        .rearrange("(c p) u -> p c u", p=128))
                    return wt

                for m in range(6, 12):
                    c = m - 6
                    wt = load_wq(m)
                    pq = ps.tile([128, 1024], F32, name="ps_qkv", tag="psA")
                    for cc in range(6):
                        nc.tensor.matmul(pq[:, 0:512], wt[:, cc, :],
                                         yT[:, cc, :], start=(cc == 0),
                                         stop=(cc == 5))
                    # even head 2c
                    act(Ka[0:64, :, 2 * c, :],
                        pq[0:64, 0:512].rearrange("p (f t) -> p f t", f=2),
                        AF.Identity, bias=bkc_sb[0:64, l, c:c + 1], scale=1.0)
                    # odd head 2c+1: shift partitions 64:128 -> 0:64 via DMA
                    shi = stg.tile([128, LOC], F32, name="shift_hi", tag="shi")
                    act(shi[64:128, :], pq[64:128, 0:512], AF.Identity,
                        bias=bkc_sb[64:128, l, c:c + 1], scale=1.0)
                    slo = stg.tile([64, LOC], F32, name="shift_lo", tag="slo")
                    dma(out=slo[0:64, :], in_=shi[64:128, :])
                    act(Ka[0:64, :, 2 * c + 1, :],
                        slo[:].rearrange("p (f t) -> p f t", f=2),
                        AF.Identity, bias=0.0, scale=1.0)
                kv_k_in = dram.tile([66, 2 * H * HALF], F32R, name="kv_k_in",
                                    tag="kin")
                kv_k_out = dram.tile([N_CORES * 66, 2 * H * HALF], F32R,
                                     name="kv_k_out", tag="kout",
                                     addr_space="Shared")
                nc.gpsimd.dma_start(out=kv_k_in[:], in_=Ka[:, :, :, :]
                                    .rearrange("p a b t -> p (a b t)"))
                nc.gpsimd.collective_compute(
                    "AllGather", ALU.bypass,
                    replica_groups=[list(range(N_CORES))],
                    ins=[kv_k_in.opt()], outs=[kv_k_out.opt()])

                if DBG and l == 0:
                    dma(out=dbg_Ka[:, :], in_=Ka[:, :, :, :].rearrange("p a b t -> p (a b t)"))
                # ===== V part, kick AG-V =====
                wv_t = []
                for c in range(6):
                    wt = wvp.tile([128, EMB], BF16, name="wv_t", tag="wv")
                    nc.gpsimd.dma_start(
                        out=wt[:], in_=wv[l, 128 * c:128 * (c + 1), :])
                    wv_t.append(wt)
                for jg in range(4):
                    half, jj = jg // 2, jg % 2
                    pv = ps.tile([128, 1024], F32, name="ps_v", tag="psA")
                    for cc in range(6):
                        nc.tensor.matmul(pv[:, 0:512],
                                         yT[:, cc, 128 * jg:128 * (jg + 1)],
                                         wv_t[cc][:, 0:512], start=(cc == 0),
                                         stop=(cc == 5))
                        nc.tensor.matmul(pv[:, 512:768],
                                         yT[:, cc, 128 * jg:128 * (jg + 1)],
                                         wv_t[cc][:, 512:768], start=(cc == 0),
                                         stop=(cc == 5))
                    vec.tensor_copy(out=Va[:, half, 0:8, jj, 0:64],
                                    in_=pv[:, 0:512].rearrange(
                                        "p (h e) -> p h e", h=8))
                    vec.tensor_copy(out=Va[:, half, 8:12, jj, 0:64],
                                    in_=pv[:, 512:768].rearrange(
                                        "p (h e) -> p h e", h=4))
                kv_v_in = dram.tile([128, 2 * H * 2 * 65], BF16, name="kv_v_in",
                                    tag="vin")
                kv_v_out = dram.tile([N_CORES * 128, 2 * H * 2 * 65], BF16,
                                     name="kv_v_out", tag="vout",
                                     addr_space="Shared")
                nc.gpsimd.dma_start(out=kv_v_in[:], in_=Va[:, :, :, :, :]
                                    .rearrange("p a b c e -> p (a b c e)"))
                nc.gpsimd.collective_compute(
                    "AllGather", ALU.bypass,
                    replica_groups=[list(range(N_CORES))],
                    ins=[kv_v_in.opt()], outs=[kv_v_out.opt()])

                # ===== QKV: q part (overlaps the collectives) =====
                for m in range(6):
                    wt = load_wq(m)
                    pq = ps.tile([128, 1024], F32, name="ps_q", tag="psA")
                    for cc in range(6):
                        nc.tensor.matmul(pq[:, 0:512], wt[:, cc, :],
                                         yT[:, cc, :], start=(cc == 0),
                                         stop=(cc == 5))
                    act(qa[2 * m][0:64, :], pq[0:64, 0:512], AF.Identity,
                        bias=bqs_sb[0:64, l, m:m + 1], scale=SCALE)
                    shi = stg.tile([128, LOC], F32, name="shift_hi", tag="shi")
                    act(shi[64:128, :], pq[64:128, 0:512], AF.Identity,
                        bias=bqs_sb[64:128, l, m:m + 1], scale=SCALE)
                    slo = stg.tile([64, LOC], F32, name="shift_lo", tag="slo")
                    dma(out=slo[0:64, :], in_=shi[64:128, :])
                    act(qa[2 * m + 1][0:64, :], slo[:], AF.Identity,
                        bias=0.0, scale=1.0)

                if DBG and l == 0:
                    nc.gpsimd.dma_start(out=dbg_yT[:, :, :], in_=yT[:])
                    dma(out=dbg_qa[:, 0:LOC], in_=qa[0][:])
                    dma(out=dbg_qa[:, LOC:2 * LOC], in_=qa[1][:])
                if PHASES == "qkv":
                    continue
                # ===== attention: 3 passes x 4 heads =====
                for g in range(3):
                    pO = [ps.tile([128, 1024], F32, name=f"ps_o{g}_{t}",
                                  tag="psA") for t in range(2)]

                    def psumO(hp):  # noqa: B023
                        return pO[hp // 2][0:65, 512 * (hp % 2):512 * (hp % 2 + 1)]

                    kv_k_view = kv_k_out[:, 1024 * g:]
                    kv_v_view = kv_v_out[:, 520 * g:]
                    for s in range(7):
                        kr = kresp.tile([68, 1024], F32R, name="kres")
                        nc.gpsimd.dma_start(
                            out=kr[0:66, :],
                            in_=kv_k_view[bass.ds(rv_krow[s], 66),
                                          bass.ds(rv_kcol[s], 1024)])
                        dma(out=kr[66:68, :], in_=flagrows[s, :, :])
                        vr = vresp.tile([128, 520], BF16, name="vres")
                        nc.gpsimd.dma_start(
                            out=vr[:],
                            in_=kv_v_view[bass.ds(rv_vrow[s], 128),
                                          bass.ds(rv_vcol[s], 520)])
                        for hp in range(4):
                            h = 4 * g + hp
                            st_t = ps.tile([128, 1024], F32, name="ps_s",
                                           tag="psA")
                            for jj in range(2):
                                nc.tensor.matmul(
                                    st_t[:, 512 * jj:512 * (jj + 1)],
                                    kr[0:68, 256 * hp + 128 * jj:
                                       256 * hp + 128 * (jj + 1)],
                                    qa[h][0:68, :], start=True, stop=True)
                            ex = expp.tile([128, 1024], BF16, name="expS")
                            act(ex[:], st_t[:], AF.Exp, bias=0.0, scale=1.0)
                            if DBG and l == 0 and g == 0 and s == 0 and hp == 0:
                                nc.gpsimd.dma_start(out=dbg_ex[:, 0:1024], in_=ex[:])
                                dma(out=dbg_kr[:, :], in_=kr[:])
                            for jj in range(2):
                                nc.tensor.matmul(
                                    psumO(hp),
                                    vr[:, 130 * hp + 65 * jj:
                                       130 * hp + 65 * (jj + 1)],
                                    ex[:, 512 * jj:512 * (jj + 1)],
                                    start=(s == 0 and jj == 0), stop=False)
                    # diagonal blocks (local, causal-masked)
                    for hp in range(4):
                        h = 4 * g + hp
                        st_t = ps.tile([128, 1024], F32, name="ps_sd", tag="psA")
                        for qc in range(2):
                            for jj in range(2):
                                nc.tensor.matmul(
                                    st_t[:, 256 * (2 * qc + jj):
                                         256 * (2 * qc + jj + 1)],
                                    Ka[0:66, qc, h, 128 * jj:128 * (jj + 1)],
                                    qa[h][0:66, 256 * qc:256 * (qc + 1)],
                                    start=True, stop=True)
                        ex = expp.tile([128, 1024], BF16, name="expS")
                        act(ex[:], st_t[:], AF.Exp, bias=0.0, scale=1.0)
                        if DBG and l == 0 and g == 0 and hp == 0:
                            nc.gpsimd.dma_start(out=dbg_ex[:, 1024:2048], in_=ex[:])
                        for qc in range(2):
                            for jj in range(2):
                                nc.gpsimd.affine_select(
                                    out=ex[:, 256 * (2 * qc + jj):
                                           256 * (2 * qc + jj + 1)],
                                    in_=ex[:, 256 * (2 * qc + jj):
                                           256 * (2 * qc + jj + 1)],
                                    pattern=[[1, 256]], compare_op=ALU.is_ge,
                                    fill=0.0, base=-128 * jj,
                                    channel_multiplier=-1)
                        for qc in range(2):
                            for jj in range(2):
                                nc.tensor.matmul(
                                    psumO(hp)[0:65, 256 * qc:256 * (qc + 1)],
                                    Va[:, qc, h, jj, 0:65],
                                    ex[:, 256 * (2 * qc + jj):
                                       256 * (2 * qc + jj + 1)],
                                    start=False,
                                    stop=(qc == 1 and jj == 1))
                    # divide + bias + write oT
                    for hp in range(4):
                        h = 4 * g + hp
                        rec = smal.tile([1, LOC], F32R, name="recip", tag="smal")
                        vec.reciprocal(out=rec[:], in_=psumO(hp)[64:65, :])
                        bc = ps.tile([128, 1024], F32, name="ps_bc", tag="psA")
                        nc.tensor.matmul(bc[0:64, 0:512], onesrow[0:1, 0:64], rec[:],
                                         start=True, stop=True)
                        tmpo = stg.tile([64, LOC], F32, name="tmpo", tag="tmpo")
                        act(tmpo[:], psumO(hp)[0:64, :], AF.Identity,
                            bias=0.0, scale=1.0)
                        vec.tensor_tensor(out=tmpo[:], in0=tmpo[:],
                                          in1=bc[0:64, 0:512], op=ALU.mult)
                        if h % 2 == 0:
                            act(oT[0:64, h // 2, :], tmpo[:], AF.Identity,
                                bias=bvh_sb[:, l, h:h + 1], scale=1.0)
                        else:
                            osg = stg.tile([64, LOC], BF16, name="ostg",
                                           tag="ostg")
                            act(osg[:], tmpo[:], AF.Identity,
                                bias=bvh_sb[:, l, h:h + 1], scale=1.0)
                            nc.gpsimd.dma_start(out=oT[64:128, h // 2, :],
                                                in_=osg[0:64, :])

                # ===== output projection + residual =====
                for m in range(6):
                    wt = wop.tile([128, 6, 128], BF16, name="wo_t", tag="wo")
                    nc.gpsimd.dma_start(
                        out=wt[:], in_=wo[l, :, 128 * m:128 * (m + 1)]
                        .rearrange("(c p) u -> p c u", p=128))
                    pp_ = ps.tile([128, 1024], F32, name="ps_proj", tag="psA")
                    for cc in range(6):
                        nc.tensor.matmul(pp_[:, 0:512], wt[:, cc, :],
                                         oT[:, cc, :], start=(cc == 0),
                                         stop=(cc == 5))
                    act(pp_[:, 0:512], pp_[:, 0:512], AF.Identity,
                        bias=boc_sb[:, l, m:m + 1], scale=1.0)
                    vec.tensor_tensor(out=hT[:, m, :], in0=hT[:, m, :],
                                      in1=pp_[:, 0:512], op=ALU.add)

                if DBG and l == 0:
                    nc.gpsimd.dma_start(out=dbg_oT[:, :, :], in_=oT[:])
                    for c in range(6):
                        ht = tmpp.tile([128, LOC], F32, name="lntmp", tag="lntmp")
                        vec.tensor_copy(out=ht[:], in_=hT[:, c, :])
                        dma(out=dbg_hT[:, c, :], in_=ht[:])
                # ===== LN2 =====
                ln_into(yT, lambda c: l2s_sb[:, l, c:c + 1],
                        lambda c: l2b_sb[:, l, c:c + 1])

                # ===== FFN =====
                pH = [ps.tile([128, 1024], F32, name=f"ps_h{t}", tag="psA")
                      for t in range(3)]
                for km in range(24):
                    wt1 = w1p.tile([128, 6, 128], BF16, name="w1_t", tag="w1")
                    nc.gpsimd.dma_start(
                        out=wt1[:], in_=w1[l, :, 128 * km:128 * (km + 1)]
                        .rearrange("(c p) u -> p c u", p=128))
                    wt2 = w2p.tile([128, EMB], BF16, name="w2_t", tag="w2")
                    nc.gpsimd.dma_start(
                        out=wt2[:], in_=w2[l, 128 * km:128 * (km + 1), :])
                    if km % 2 == 0:
                        pF = ps.tile([128, 1024], F32, name="ps_f", tag="psA")
                    half = 512 * (km % 2)
                    for cc in range(6):
                        nc.tensor.matmul(pF[:, half:half + 512], wt1[:, cc, :],
                                         yT[:, cc, :], start=(cc == 0),
                                         stop=(cc == 5))
                    g_sb = gp.tile([128, 512], BF16, name="g_sb")
                    act(g_sb[:], pF[:, half:half + 512], AF.Gelu_apprx_tanh,
                        bias=0.0, scale=1.0)
                    for m in range(6):
                        nc.tensor.matmul(
                            pH[m // 2][:, 512 * (m % 2):512 * (m % 2 + 1)],
                            wt2[:, 128 * m:128 * (m + 1)], g_sb[:],
                            start=(km == 0), stop=(km == 23))
                for m in range(6):
                    vec.tensor_tensor(
                        out=hT[:, m, :], in0=hT[:, m, :],
                        in1=pH[m // 2][:, 512 * (m % 2):512 * (m % 2 + 1)],
                        op=ALU.add)

            # ===== final LN -> output =====
            out_sb = pp.tile([128, 6, LOC], F32, name="out_sb")
            ln_into(out_sb, lambda c: lnf_sb[:, 0, c:c + 1],
                    lambda c: lnf_sb[:, 1, c:c + 1])
            dma(out=out_d[:, :, :], in_=out_sb[:])

    nc.compile()
    _CACHED["nc"] = nc
    return nc


# ---------------------------------------------------------------------------
# host side
# ---------------------------------------------------------------------------


def _host_prep(x, wqkv, bqkv, wo, bo, ln1s, ln1b, ln2s, ln2b, w1, w2, lnfs,
               lnfb):
    """Build the 8 per-core input maps."""
    d = _DEPTH
    w_qk = np.ascontiguousarray(wqkv[:d, :, :2 * EMB]).astype(bf16)
    w_v = np.ascontiguousarray(wqkv[:d, :, 2 * EMB:]).astype(bf16)
    wo_b = wo[:d].astype(bf16)
    w1_b = w1[:d].astype(bf16)
    w2_b = w2[:d].astype(bf16)

    def cols(v):  # [d, 768] -> [128, d, 6]
        return np.ascontiguousarray(
            v.reshape(d, 6, 128).transpose(2, 0, 1)).astype(np.float32)

    bq = cols(bqkv[:d, 0:EMB] * SCALE)
    bk = cols(bqkv[:d, EMB:2 * EMB])
    bo_t = cols(bo[:d])
    bv_head = np.ascontiguousarray(
        bqkv[:d, 2 * EMB:].reshape(d, H, 64).transpose(2, 0, 1)
    ).astype(np.float32)  # [64, d, H]
    l1s, l1b = cols(ln1s[:d]), cols(ln1b[:d])
    l2s, l2b = cols(ln2s[:d]), cols(ln2b[:d])
    lnf = np.stack([lnfs.reshape(6, 128).T, lnfb.reshape(6, 128).T],
                   axis=1).astype(np.float32)  # [128, 2, 6]

    qkill_a = np.zeros((2, LOC), np.float32)
    qkill_a[0, HALF:] = KILL
    qkill_a[1, :HALF] = KILL

    in_maps = []
    for i in range(N_CORES):
        xs = np.concatenate(
            [x[0, HALF * i:HALF * (i + 1)],
             x[1, HALF * (7 - i):HALF * (8 - i)]], axis=0)  # [512, 768]
        xT_t = np.ascontiguousarray(
            xs.T.reshape(6, 128, LOC).transpose(1, 0, 2)).astype(np.float32)

        qpos = np.concatenate([HALF * i + np.arange(HALF),
                               HALF * (7 - i) + np.arange(HALF)]).astype(
                                   np.float64)
        qaug_a = np.empty((H, 2, LOC), np.float32)
        for h in range(H):
            qaug_a[h, 0, :] = SLOPES[h]
            qaug_a[h, 1, :] = -SLOPES[h] * qpos
        kaug_a = np.stack([qpos.astype(np.float32),
                           np.ones(LOC, np.float32)], axis=0)

        # slots: s < i -> (r=s, half=0); s >= i -> (r=s+1, half=1)
        krow = np.empty(7, np.int32); kcol = np.empty(7, np.int32)
        vrow = np.empty(7, np.int32); vcol = np.empty(7, np.int32)
        flag = np.zeros((7, 2, 1024), np.float32)
        for s in range(7):
            if s < i:
                r, halfsel = s, 0
            else:
                r, halfsel = s + 1, 1
            krow[s] = 66 * r
            kcol[s] = 3072 * halfsel
            vrow[s] = 128 * r
            vcol[s] = 1560 * halfsel
            flag[s, 0, :] = 1.0 if halfsel == 0 else 0.0
            flag[s, 1, :] = 0.0 if halfsel == 0 else 1.0
        sreg = np.concatenate([krow, kcol, vrow, vcol]).reshape(1, 28)

        in_maps.append({
            "xT": xT_t, "wqk": w_qk, "wv": w_v, "wo": wo_b, "w1": w1_b,
            "w2": w2_b, "bq_s": bq, "bk_c": bk, "bo_c": bo_t, "bv_h": bv_head,
            "ln1s_c": l1s, "ln1b_c": l1b, "ln2s_c": l2s, "ln2b_c": l2b,
            "lnf_c": lnf, "qaug": qaug_a, "qkill": qkill_a, "kaug": kaug_a,
            "flagrows": flag, "slotreg": sreg,
            "ones_col": np.ones((128, 1), np.float32),
            "ones_row": np.concatenate([np.ones(128, np.float32),
                                        -np.ones(128, np.float32)])[None, :],
        })
    return in_maps


_LAST_RESULTS = {}


def _run_device(in_maps):
    from concourse.bass_utils import run_bass_kernel_spmd
    nc = _build()
    trace = bool(int(os.environ.get("BASS_KERNEL_TRACE", "0")))
    try:
        res = run_bass_kernel_spmd(nc, in_maps, list(range(N_CORES)),
                                   trace=trace)
    except (ImportError, ModuleNotFoundError):
        res = run_bass_kernel_spmd(nc, in_maps, list(range(N_CORES)),
                                   trace=False)
    _LAST_RESULTS["res"] = res
    return res.results


def kernel(x, wqkv, bqkv, wo, bo, ln1s, ln1b, ln2s, ln2b, w1, w2, lnfs, lnfb):
    x = np.asarray(x, np.float32)
    args = [np.asarray(a, np.float32) for a in
            (wqkv, bqkv, wo, bo, ln1s, ln1b, ln2s, ln2b, w1, w2, lnfs, lnfb)]
    in_maps = _host_prep(x, *args)
    results = _run_device(in_maps)
    out = np.empty((B, N, EMB), np.float32)
    for i in range(N_CORES):
        o = np.asarray(results[i]["out"], np.float32)  # [128, 6, 512]
        oT = o.transpose(1, 0, 2).reshape(EMB, LOC)    # [768, 512]
        out[0, HALF * i:HALF * (i + 1)] = oT[:, :HALF].T
        out[1, HALF * (7 - i):HALF * (8 - i)] = oT[:, HALF:].T
    return out
